# revision 30
# baseline (speedup 1.0000x reference)
"""Trainium2 Bass kernel for nn_BiLSTMCell (graph-LSTM cell).

Math (per batch row):
    g_pre[g] = x @ Wx[g].T + hidden @ Wh[g].T + neighbors @ Wn[g].T + b[g]
    i, f, o = sigmoid(g_pre[0..2]);  s = tanh(g_pre[3])
    next_cell = f * cell + i * s
    next_hidden = o * tanh(next_cell)

Strategy: data-parallel over the batch (8192 -> 1024 rows/core on 8 cores),
weights replicated. The x/hidden operands are fused on host into one
A = [x | hidden] with K = 2048 = 16*128, so each gate pre-activation is a
single 16-step accumulating PE matmul chain:
    g_pre[g]^T = W_all[g] @ A^T      ([128k,128h]^T @ [128k,512b] per step)
in fp16 (f32 PSUM accumulate; fp16 enables Fast Weight Load so the
128-cycle LDWEIGHTS hides under the 512-cycle stream, unlike f32r which
paid it serially -- ~280ns/matmul -> ~220ns/matmul).

The rank-4 neighbor term (neighbors @ Wn[g].T, 0.27 GFLOP) is computed on
the host and shipped as an f32 addend; it joins the pre-activation via one
VectorE add per gate. The bias rides the ScalarE activation's per-partition
bias port for free. This keeps the PE stream at exactly 1024 matmuls/core.

Outputs are produced transposed/tiled and unscrambled on the host.
"""

import os
import sys

import numpy as np


def _import_concourse():
    try:
        import concourse.bass  # noqa: F401
        return
    except ImportError:
        pass
    for p in ("/opt/trn_rl_repo", "/root/.axon_site/_ro/trn_rl_repo"):
        if os.path.isdir(p) and p not in sys.path:
            sys.path.insert(0, p)
    import concourse.bass  # noqa: F401


B, IN, H, NB, G = 8192, 1024, 1024, 4, 4
NCORES = 8
BS = B // NCORES        # 1024 batch rows per core
KT = 16                 # k-tiles of 128 (IN + H = 2048)
HT = H // 128           # 8 h-tiles of 128
BT = BS // 512          # 2 b-tiles of 512


def _split_excess_waits(nc, max_waits=1, drain_max=0):
    """This walrus build's codegen supports very few sync-wait commands per
    instruction (1 for most ops, 0 spare on Drain). Hoist excess sem-waits
    onto preceding wait-only NoOps on the same engine (AND-semantics over
    monotone semaphores makes sequential waiting equivalent)."""
    from concourse import mybir

    uid = [0]
    n_split = 0
    for fn in nc.m.functions:
        for bb in fn.blocks:
            new_insts = []
            for inst in bb.instructions:
                limit = drain_max if type(inst).__name__ == "InstDrain" else max_waits
                si = inst.sync_info
                waits = list(si.on_wait) if si and si.on_wait else []
                if len(waits) > limit:
                    n_split += 1
                    if limit > 0:
                        excess, keep = waits[:-limit], waits[-limit:]
                    else:
                        excess, keep = waits, []
                    for i in range(0, len(excess), max_waits):
                        chunk = excess[i:i + max_waits]
                        nop = mybir.InstNoOp(
                            name=f"waitsplit_{uid[0]}",
                            sync_info=mybir.SyncInfo(on_wait=chunk, on_update=[]),
                        )
                        uid[0] += 1
                        nop.engine = inst.engine
                        new_insts.append(nop)
                    si.on_wait = keep
                    inst.sync_info = si
                new_insts.append(inst)
            bb.instructions = new_insts
    return n_split


def _dedupe_ldweights(nc):
    """Our bb-paired emission produces [LDW_a, MM(b0), LDW_b, MM(b1)] with
    LDW_a == LDW_b (identical weight AP). The PE only commits freshly loaded
    weights after the in-flight matmul fully drains (~110 cycles), so every
    redundant reload costs ~46ns. Deleting LDW_b lets MM(b1) stream against
    the already-committed weights back-to-back. Waits on a deleted LDW are
    merged into the following matmul (excess waits are split later by
    _split_excess_waits)."""
    from concourse import mybir

    n_del = 0
    for fn in nc.m.functions:
        for bb in fn.blocks:
            insts = bb.instructions
            new_insts = []
            last_ldw_key = None
            pending_waits = []
            for inst in insts:
                tname = type(inst).__name__
                if tname == "InstLdweights":
                    key = str(inst.ins[0])
                    if key == last_ldw_key:
                        si = inst.sync_info
                        if si and si.on_wait:
                            pending_waits.extend(si.on_wait)
                        n_del += 1
                        continue
                    last_ldw_key = key
                    new_insts.append(inst)
                elif tname == "InstMatmult":
                    if pending_waits:
                        si = inst.sync_info or mybir.SyncInfo(
                            on_wait=[], on_update=[]
                        )
                        si.on_wait = list(si.on_wait) + pending_waits
                        inst.sync_info = si
                        pending_waits = []
                    new_insts.append(inst)
                else:
                    # any other PE-visible instruction invalidates the cache
                    if getattr(inst, "engine", None) == mybir.EngineType.PE:
                        last_ldw_key = None
                    new_insts.append(inst)
            assert not pending_waits
            bb.instructions = new_insts
    return n_del


_PROG = None


def _build_program():
    import concourse.bass as bass
    import concourse.tile as tile
    from concourse import mybir

    f32 = mybir.dt.float32
    f16 = mybir.dt.float16
    ACT = mybir.ActivationFunctionType

    nc = bass.Bass()
    at_d = nc.dram_tensor("AT", [128, KT, BS], f16, kind="ExternalInput")
    w_d = nc.dram_tensor("W", [HT, 128, KT, G * 128], f16, kind="ExternalInput")
    ct_d = nc.dram_tensor("CT", [HT, BT, 128, 512], f16, kind="ExternalInput")
    nb_d = nc.dram_tensor("NBT", [HT, BT, 128, G, 512], f16, kind="ExternalInput")
    bias_d = nc.dram_tensor("BIAS", [128, HT * G + 1], f32, kind="ExternalInput")
    ho_d = nc.dram_tensor("hT", [HT, BT, 128, 512], f16, kind="ExternalOutput")
    co_d = nc.dram_tensor("cT", [HT, BT, 128, 512], f16, kind="ExternalOutput")

    with tile.TileContext(nc) as tc:
        with (
            tc.tile_pool(name="at", bufs=1) as p_at,
            tc.tile_pool(name="w", bufs=3) as p_w,
            tc.tile_pool(name="cell", bufs=3) as p_cell,
            tc.tile_pool(name="nb", bufs=2) as p_nb,
            tc.tile_pool(name="bias", bufs=1) as p_bias,
            tc.tile_pool(name="eps", bufs=2) as p_eps,
            tc.tile_pool(name="outs", bufs=2) as p_out,
            tc.tile_pool(name="ps", bufs=8, space="PSUM") as p_ps,
        ):
            bias_t = p_bias.tile([128, HT * G + 1], f32, name="bias_t")
            nc.gpsimd.dma_start(bias_t[:], bias_d[:])
            at = p_at.tile([128, KT, BS], f16, name="at")

            # PE warmup: dummy matmuls with no DMA deps run during the DMA
            # head and un-throttle the HAM, so the real stream starts at the
            # warm 216ns/mm rate instead of ramping over its first ~100 mms.
            # warm_ps shares the psum ring; it recycles once h0's eighth
            # chain starts, long after the warmups retire.
            warm = p_eps.tile([128, 128], f16, name="warm", tag="warm")
            nc.gpsimd.memset(warm[:], 0.0)
            warm_ps = p_ps.tile([128, 128], f32, name="warm_ps", tag="ps")
            for _ in range(48):
                nc.tensor.matmul(warm_ps[:], warm[:], warm[:], start=True,
                                 stop=True)

            # One queue saturates HBM (~390 GB/s), so multi-queue splits only
            # reduce each stream's share. Put ALL bulk traffic (W, AT) on the
            # sync queue in exact consumption order; cell/neighbor ride the
            # scalar queue, outputs get gpsimd to themselves.
            wts = []
            for hh in range(HT):
                wts.append(p_w.tile([128, KT, G * 128], f16, name="wt", tag="wt"))

            # head: every dma_start costs ~0.7us of SEQUENCER issue time
            # (DIRECT2D descriptor gen), so the head is issue-rate limited as
            # well as bandwidth limited. The h0 g3-block consumes W0+AT
            # (6.3MB) in ~7us -- more than one queue's bandwidth -- so
            # round-robin the chunks across the sync AND scalar queues in
            # exact consumption order (~3.15MB each), chunks coarsening as
            # the pipeline gets ahead.
            spans = [(0, 1), (1, 2), (2, 3), (3, 4), (4, 6), (6, 8),
                     (8, 12), (12, 16)]
            head = []
            for lo, hi in spans:
                head.append(("w", lo, hi))
                head.append(("a", lo, hi))
            for i, (kind, lo, hi) in enumerate(head):
                q = nc.sync if i % 2 == 0 else nc.scalar
                if kind == "w":
                    q.dma_start(wts[0][:, lo:hi, :], w_d[0, :, lo:hi, :])
                else:
                    q.dma_start(at[:, lo:hi, :], at_d[:, lo:hi, :])
            for hh in range(1, HT):
                nc.sync.dma_start(wts[hh][:], w_d[hh])

            for hh in range(HT):
                wt = wts[hh]

                cts, nbts = [], []
                for bb in range(BT):
                    ct = p_cell.tile([128, 512], f16, name="ct", tag="ct")
                    nc.scalar.dma_start(ct[:], ct_d[hh, bb])
                    cts.append(ct)
                    nbt = p_nb.tile([128, G, 512], f16, name="nbt", tag="nbt")
                    nc.scalar.dma_start(nbt[:], nb_d[hh, bb])
                    nbts.append(nbt)

                # Both b-tiles processed together so each weight tile is
                # loaded ONCE and streamed against b0 then b1: the second
                # matmul of a pair needs no weight-slot commit, so its fill
                # overlaps the first's drain (weight commit requires a fully
                # drained array -> 259 ns/mm when every mm reloads weights).
                # Gates run as sequential blocks (s,i,f,o) so the s-block's
                # psum banks free ~75% before the h-tile's stream ends and
                # the next h-tile never waits on bank recycling.
                ps = [[None] * G for _ in range(BT)]
                for g in (3, 0, 1, 2):
                    for bb in range(BT):
                        ps[bb][g] = p_ps.tile(
                            [128, 512], f32, name=f"pt{g}_{bb}", tag="ps"
                        )
                    for kk in range(KT):
                        for bb in range(BT):
                            nc.tensor.matmul(
                                ps[bb][g][:],
                                wt[:, kk, g * 128:(g + 1) * 128],
                                at[:, kk, bb * 512:(bb + 1) * 512],
                                start=(kk == 0),
                                stop=(kk == KT - 1),
                            )

                bcol = lambda g: bias_t[:, hh * G + g:hh * G + g + 1]
                last_tile = hh == HT - 1

                def pre(g, bb, name):
                    # pre-activation = psum + neighbor term (bias joins via
                    # the ACT bias port)
                    t = p_eps.tile([128, 512], f32, name=name, tag=f"{name}{bb}")
                    nc.vector.tensor_add(t[:], ps[bb][g][:], nbts[bb][:, g, :])
                    return t

                # bb0/bb1 interleaved per gate: DVE/ACT stay FIFO-pipelined
                # and each psum bank frees as early as its data allows.
                tan_s = [pre(3, bb, "tan_s") for bb in range(BT)]
                for bb in range(BT):
                    nc.scalar.activation(tan_s[bb][:], tan_s[bb][:], ACT.Tanh,
                                         bias=bcol(3))
                sig_i = [pre(0, bb, "sig_i") for bb in range(BT)]
                for bb in range(BT):
                    nc.scalar.activation(sig_i[bb][:], sig_i[bb][:], ACT.Sigmoid,
                                         bias=bcol(0))
                sig_f = [pre(1, bb, "sig_f") for bb in range(BT)]
                for bb in range(BT):
                    nc.scalar.activation(sig_f[bb][:], sig_f[bb][:], ACT.Sigmoid,
                                         bias=bcol(1))

                c_news, tan_cs = [], []
                for bb in range(BT):
                    t_is = p_eps.tile([128, 512], f32, name="t_is", tag=f"t_is{bb}")
                    nc.vector.tensor_mul(t_is[:], sig_i[bb][:], tan_s[bb][:])
                    t_fc = p_eps.tile([128, 512], f32, name="t_fc", tag=f"t_fc{bb}")
                    nc.vector.tensor_mul(t_fc[:], sig_f[bb][:], cts[bb][:])
                    c_new = p_out.tile([128, 512], f16, name="c_new", tag=f"c_new{bb}")
                    nc.vector.tensor_add(c_new[:], t_is[:], t_fc[:])
                    c_news.append(c_new)
                    tan_c = p_eps.tile([128, 512], f32, name="tan_c", tag=f"tan_c{bb}")
                    # explicit zero-bias AP: a float bias would make the
                    # framework stage a const tensor via a TENSOR_LOAD that
                    # delays the sync queue's first DMA at the critical head
                    nc.scalar.activation(tan_c[:], c_new[:], ACT.Tanh,
                                         bias=bias_t[:, HT * G:HT * G + 1])
                    tan_cs.append(tan_c)
                    # c outputs are ready before the o-gate stream ends; the
                    # last tile's ride the sync queue (idle by then) so their
                    # issue cost doesn't delay the scalar queue's sig_o
                    qc = nc.sync if last_tile else nc.gpsimd
                    qc.dma_start(co_d[hh, bb][:], c_new[:])

                if not last_tile:
                    sig_o = [pre(2, bb, "sig_o") for bb in range(BT)]
                    for bb in range(BT):
                        nc.scalar.activation(sig_o[bb][:], sig_o[bb][:],
                                             ACT.Sigmoid, bias=bcol(2))
                    for bb in range(BT):
                        h_new = p_out.tile([128, 512], f16, name="h_new",
                                           tag=f"h_new{bb}")
                        nc.vector.tensor_mul(h_new[:], sig_o[bb][:], tan_cs[bb][:])
                        nc.gpsimd.dma_start(ho_d[hh, bb][:], h_new[:])
                else:
                    # tail: the o-gate path gates the end of the kernel, so
                    # run it in column halves -- each half's h output flushes
                    # (on the idle sync queue) while the next half computes
                    for bb in range(BT):
                        for lo, hi in ((0, 256), (256, 512)):
                            sl = slice(lo, hi)
                            t = p_eps.tile([128, hi - lo], f32, name="sig_o",
                                           tag=f"sig_o{bb}_{lo}")
                            nc.vector.tensor_add(t[:], ps[bb][2][:, sl],
                                                 nbts[bb][:, 2, sl])
                            nc.scalar.activation(t[:], t[:], ACT.Sigmoid,
                                                 bias=bcol(2))
                            h_new = p_out.tile([128, hi - lo], f16, name="h_new",
                                               tag=f"h_new{bb}_{lo}")
                            nc.vector.tensor_mul(h_new[:], t[:], tan_cs[bb][:, sl])
                            nc.sync.dma_start(ho_d[hh, bb][:, sl], h_new[:])

    _dedupe_ldweights(nc)
    _split_excess_waits(nc)
    return nc


def _get_program():
    global _PROG
    if _PROG is None:
        _PROG = _build_program()
    return _PROG


def _prep_inputs(x, hidden, cell, neighbors, Wx, Wh, Wn, b):
    """Host-side shard/relayout. Returns per-core input maps."""
    x = np.asarray(x, np.float32)
    hidden = np.asarray(hidden, np.float32)
    cell = np.asarray(cell, np.float32)
    neighbors = np.asarray(neighbors, np.float32)
    Wx = np.asarray(Wx, np.float32)
    Wh = np.asarray(Wh, np.float32)
    Wn = np.asarray(Wn, np.float32)
    b = np.asarray(b, np.float32)

    # A = [x | hidden]: K = 2048 exactly.
    A = np.concatenate([x, hidden], axis=1)
    W_all = np.concatenate([Wx, Wh], axis=2)  # [G, H, 2048]

    # SBUF weight layout: [hh, p(k), kk, g*128 + j(h)], fp16
    w_host = np.ascontiguousarray(
        W_all.reshape(G, HT, 128, KT, 128).transpose(1, 4, 3, 0, 2)
    ).reshape(HT, 128, KT, G * 128).astype(np.float16)

    # neighbor term, [B, G, H] computed on host in f64 -> f32
    nbterm = np.einsum(
        "bj,ghj->gbh", neighbors.astype(np.float64), Wn.astype(np.float64)
    ).astype(np.float32)

    # bias layout [j, hh*G + g] = b[g, hh*128+j]
    bias_host = np.zeros((128, HT * G + 1), np.float32)
    bias_host[:, :HT * G] = b.reshape(G, HT, 128).transpose(2, 1, 0).reshape(128, HT * G)

    in_maps = []
    for c in range(NCORES):
        sl = slice(c * BS, (c + 1) * BS)
        # A^T tiled: [p(k), kk, b], fp16
        at_host = np.ascontiguousarray(
            A[sl].T.reshape(KT, 128, BS).transpose(1, 0, 2)
        ).astype(np.float16)
        # cell^T tiled: [hh, bb, j(h), n(b)], fp16
        ct_host = np.ascontiguousarray(
            cell[sl].T.reshape(HT, 128, BT, 512).transpose(0, 2, 1, 3)
        ).astype(np.float16)
        # neighbor term tiled: [hh, bb, j(h), g, n(b)], fp16
        nb_host = np.ascontiguousarray(
            nbterm[:, sl, :].transpose(2, 1, 0)  # [H, BS, G]
            .reshape(HT, 128, BT, 512, G)
            .transpose(0, 2, 1, 4, 3)            # [hh, bb, j, g, n]
        ).astype(np.float16)
        in_maps.append(
            {
                "AT": at_host,
                "W": w_host,
                "CT": ct_host,
                "NBT": nb_host,
                "BIAS": bias_host,
            }
        )
    return in_maps


def _gather_outputs(results):
    """Invert the per-core [HT, BT, 128, 512] transposed tiling."""
    h_parts, c_parts = [], []
    for c in range(NCORES):
        hT = np.asarray(results[c]["hT"]).astype(np.float32)
        cT = np.asarray(results[c]["cT"]).astype(np.float32)
        # [hh, bb, j, n] -> [hh*128+j, bb*512+n] -> transpose to [b, h]
        h_parts.append(hT.transpose(0, 2, 1, 3).reshape(H, BS).T)
        c_parts.append(cT.transpose(0, 2, 1, 3).reshape(H, BS).T)
    next_hidden = np.ascontiguousarray(np.concatenate(h_parts, axis=0), dtype=np.float32)
    next_cell = np.ascontiguousarray(np.concatenate(c_parts, axis=0), dtype=np.float32)
    return next_hidden, next_cell


def _run(in_maps, trace=False, tmpdir=None):
    _import_concourse()
    from concourse.bass_utils import run_bass_kernel_spmd

    if trace:
        _install_ntff_shim()
    nc = _get_program()
    last_err = None
    for attempt in range(3):
        try:
            return run_bass_kernel_spmd(
                nc, in_maps, list(range(NCORES)), trace=trace, tmpdir=tmpdir
            )
        except Exception as e:  # transient device wedge: retry
            last_err = e
            if "UNRECOVERABLE" not in str(e) and "NRT" not in str(e):
                raise
    raise last_err


def _install_ntff_shim():
    """Shim antenv.axon_hooks (absent in this image) so trace=True works."""
    import types

    if "antenv.axon_hooks" not in sys.modules:
        mod = types.ModuleType("antenv.axon_hooks")
        mod._hook = None
        mod.set_axon_ntff_profile_hook = lambda h: setattr(mod, "_hook", h)
        mod.get_axon_ntff_profile_hook = lambda: mod._hook
        sys.modules["antenv.axon_hooks"] = mod
        try:
            import antenv
            antenv.axon_hooks = mod
        except ImportError:
            pass
    mod = sys.modules["antenv.axon_hooks"]
    if mod._hook is None:
        from trn_agent_boot.trn_boot import _ntff_profile_via_ctypes
        mod._hook = _ntff_profile_via_ctypes("/opt/axon/libaxon_pjrt.so")
    from concourse import bass_utils
    bass_utils.upload_artifacts = lambda tmpdir: f"local:{tmpdir}"


def kernel(x, hidden, cell, neighbors, Wx, Wh, Wn, b):
    _import_concourse()
    in_maps = _prep_inputs(x, hidden, cell, neighbors, Wx, Wh, Wn, b)
    res = _run(in_maps, trace=False)
    return _gather_outputs(res.results)



# revision 31
# speedup vs baseline: 1.0188x; 1.0188x over previous
"""Trainium2 Bass kernel for nn_BiLSTMCell (graph-LSTM cell).

Math (per batch row):
    g_pre[g] = x @ Wx[g].T + hidden @ Wh[g].T + neighbors @ Wn[g].T + b[g]
    i, f, o = sigmoid(g_pre[0..2]);  s = tanh(g_pre[3])
    next_cell = f * cell + i * s
    next_hidden = o * tanh(next_cell)

Strategy: data-parallel over the batch (8192 -> 1024 rows/core on 8 cores),
weights replicated. The x/hidden operands are fused on host into one
A = [x | hidden] with K = 2048 = 16*128, so each gate pre-activation is a
single 16-step accumulating PE matmul chain:
    g_pre[g]^T = W_all[g] @ A^T      ([128k,128h]^T @ [128k,512b] per step)
in fp16 (f32 PSUM accumulate; fp16 enables Fast Weight Load so the
128-cycle LDWEIGHTS hides under the 512-cycle stream, unlike f32r which
paid it serially -- ~280ns/matmul -> ~220ns/matmul).

The rank-4 neighbor term (neighbors @ Wn[g].T, 0.27 GFLOP) is computed on
the host and shipped as an f32 addend; it joins the pre-activation via one
VectorE add per gate. The bias rides the ScalarE activation's per-partition
bias port for free. This keeps the PE stream at exactly 1024 matmuls/core.

Outputs are produced transposed/tiled and unscrambled on the host.
"""

import os
import sys

import numpy as np


def _import_concourse():
    try:
        import concourse.bass  # noqa: F401
        return
    except ImportError:
        pass
    for p in ("/opt/trn_rl_repo", "/root/.axon_site/_ro/trn_rl_repo"):
        if os.path.isdir(p) and p not in sys.path:
            sys.path.insert(0, p)
    import concourse.bass  # noqa: F401


B, IN, H, NB, G = 8192, 1024, 1024, 4, 4
NCORES = 8
BS = B // NCORES        # 1024 batch rows per core
KT = 16                 # k-tiles of 128 (IN + H = 2048)
HT = H // 128           # 8 h-tiles of 128
BT = BS // 512          # 2 b-tiles of 512


def _split_excess_waits(nc, max_waits=1, drain_max=0):
    """This walrus build's codegen supports very few sync-wait commands per
    instruction (1 for most ops, 0 spare on Drain). Hoist excess sem-waits
    onto preceding wait-only NoOps on the same engine (AND-semantics over
    monotone semaphores makes sequential waiting equivalent)."""
    from concourse import mybir

    uid = [0]
    n_split = 0
    for fn in nc.m.functions:
        for bb in fn.blocks:
            new_insts = []
            for inst in bb.instructions:
                limit = drain_max if type(inst).__name__ == "InstDrain" else max_waits
                si = inst.sync_info
                waits = list(si.on_wait) if si and si.on_wait else []
                if len(waits) > limit:
                    n_split += 1
                    if limit > 0:
                        excess, keep = waits[:-limit], waits[-limit:]
                    else:
                        excess, keep = waits, []
                    for i in range(0, len(excess), max_waits):
                        chunk = excess[i:i + max_waits]
                        nop = mybir.InstNoOp(
                            name=f"waitsplit_{uid[0]}",
                            sync_info=mybir.SyncInfo(on_wait=chunk, on_update=[]),
                        )
                        uid[0] += 1
                        nop.engine = inst.engine
                        new_insts.append(nop)
                    si.on_wait = keep
                    inst.sync_info = si
                new_insts.append(inst)
            bb.instructions = new_insts
    return n_split


def _dedupe_ldweights(nc):
    """Our bb-paired emission produces [LDW_a, MM(b0), LDW_b, MM(b1)] with
    LDW_a == LDW_b (identical weight AP). The PE only commits freshly loaded
    weights after the in-flight matmul fully drains (~110 cycles), so every
    redundant reload costs ~46ns. Deleting LDW_b lets MM(b1) stream against
    the already-committed weights back-to-back. Waits on a deleted LDW are
    merged into the following matmul (excess waits are split later by
    _split_excess_waits)."""
    from concourse import mybir

    n_del = 0
    for fn in nc.m.functions:
        for bb in fn.blocks:
            insts = bb.instructions
            new_insts = []
            last_ldw_key = None
            pending_waits = []
            for inst in insts:
                tname = type(inst).__name__
                if tname == "InstLdweights":
                    key = str(inst.ins[0])
                    if key == last_ldw_key:
                        si = inst.sync_info
                        if si and si.on_wait:
                            pending_waits.extend(si.on_wait)
                        n_del += 1
                        continue
                    last_ldw_key = key
                    new_insts.append(inst)
                elif tname == "InstMatmult":
                    if pending_waits:
                        si = inst.sync_info or mybir.SyncInfo(
                            on_wait=[], on_update=[]
                        )
                        si.on_wait = list(si.on_wait) + pending_waits
                        inst.sync_info = si
                        pending_waits = []
                    new_insts.append(inst)
                else:
                    # any other PE-visible instruction invalidates the cache
                    if getattr(inst, "engine", None) == mybir.EngineType.PE:
                        last_ldw_key = None
                    new_insts.append(inst)
            assert not pending_waits
            bb.instructions = new_insts
    return n_del


_PROG = None


def _build_program():
    import concourse.bass as bass
    import concourse.tile as tile
    from concourse import mybir

    f32 = mybir.dt.float32
    f16 = mybir.dt.float16
    ACT = mybir.ActivationFunctionType

    nc = bass.Bass()
    at_d = nc.dram_tensor("AT", [128, KT, BS], f16, kind="ExternalInput")
    w_d = nc.dram_tensor("W", [HT, 128, KT, G * 128], f16, kind="ExternalInput")
    ct_d = nc.dram_tensor("CT", [HT, BT, 128, 512], f16, kind="ExternalInput")
    nb_d = nc.dram_tensor("NBT", [HT, BT, 128, G, 512], f16, kind="ExternalInput")
    bias_d = nc.dram_tensor("BIAS", [128, HT * G + 1], f32, kind="ExternalInput")
    ho_d = nc.dram_tensor("hT", [HT, BT, 128, 512], f16, kind="ExternalOutput")
    co_d = nc.dram_tensor("cT", [HT, BT, 128, 512], f16, kind="ExternalOutput")

    with tile.TileContext(nc) as tc:
        with (
            tc.tile_pool(name="at", bufs=1) as p_at,
            tc.tile_pool(name="w", bufs=3) as p_w,
            tc.tile_pool(name="cell", bufs=3) as p_cell,
            tc.tile_pool(name="nb", bufs=2) as p_nb,
            tc.tile_pool(name="bias", bufs=1) as p_bias,
            tc.tile_pool(name="eps", bufs=2) as p_eps,
            tc.tile_pool(name="outs", bufs=2) as p_out,
            tc.tile_pool(name="ps", bufs=8, space="PSUM") as p_ps,
        ):
            bias_t = p_bias.tile([128, HT * G + 1], f32, name="bias_t")
            nc.gpsimd.dma_start(bias_t[:], bias_d[:])
            at = p_at.tile([128, KT, BS], f16, name="at")

            # PE warmup: dummy matmuls with no DMA deps run during the DMA
            # head and un-throttle the HAM, so the real stream starts at the
            # warm 216ns/mm rate instead of ramping over its first ~100 mms.
            # warm_ps shares the psum ring; it recycles once h0's eighth
            # chain starts, long after the warmups retire.
            warm = p_eps.tile([128, 128], f16, name="warm", tag="warm")
            nc.gpsimd.memset(warm[:], 0.0)
            warm_ps = p_ps.tile([128, 128], f32, name="warm_ps", tag="ps")
            for _ in range(48):
                nc.tensor.matmul(warm_ps[:], warm[:], warm[:], start=True,
                                 stop=True)

            # One queue saturates HBM (~390 GB/s), so multi-queue splits only
            # reduce each stream's share. Put ALL bulk traffic (W, AT) on the
            # sync queue in exact consumption order; cell/neighbor ride the
            # scalar queue, outputs get gpsimd to themselves.
            wts = []
            for hh in range(HT):
                wts.append(p_w.tile([128, KT, G * 128], f16, name="wt", tag="wt"))

            # head: every dma_start costs ~0.8us of SEQUENCER issue time
            # (DIRECT2D descriptor gen), so the head is issue-rate limited,
            # not bandwidth limited. Split W0 (sync queue) and AT (scalar
            # queue) so the two issue streams run in parallel, and coarsen
            # chunks progressively: small chunks up front so the PE starts
            # early, big chunks behind to cut issue count.
            w0_chunks = [(0, 1), (1, 2), (2, 3), (3, 4),
                         (4, 6), (6, 8), (8, 12), (12, 16)]
            at_chunks = [(0, 1), (1, 2), (2, 4), (4, 6),
                         (6, 8), (8, 12), (12, 16)]
            for lo, hi in w0_chunks:
                nc.sync.dma_start(wts[0][:, lo:hi, :], w_d[0, :, lo:hi, :])
            for lo, hi in at_chunks:
                nc.scalar.dma_start(at[:, lo:hi, :], at_d[:, lo:hi, :])
            for hh in range(1, HT):
                nc.sync.dma_start(wts[hh][:], w_d[hh])

            for hh in range(HT):
                wt = wts[hh]

                cts, nbts = [], []
                for bb in range(BT):
                    ct = p_cell.tile([128, 512], f16, name="ct", tag="ct")
                    nc.scalar.dma_start(ct[:], ct_d[hh, bb])
                    cts.append(ct)
                    nbt = p_nb.tile([128, G, 512], f16, name="nbt", tag="nbt")
                    nc.scalar.dma_start(nbt[:], nb_d[hh, bb])
                    nbts.append(nbt)

                # Both b-tiles processed together so each weight tile is
                # loaded ONCE and streamed against b0 then b1: the second
                # matmul of a pair needs no weight-slot commit, so its fill
                # overlaps the first's drain (weight commit requires a fully
                # drained array -> 259 ns/mm when every mm reloads weights).
                # Gates run as sequential blocks (s,i,f,o) so the s-block's
                # psum banks free ~75% before the h-tile's stream ends and
                # the next h-tile never waits on bank recycling.
                ps = [[None] * G for _ in range(BT)]
                for g in (3, 0, 1, 2):
                    for bb in range(BT):
                        ps[bb][g] = p_ps.tile(
                            [128, 512], f32, name=f"pt{g}_{bb}", tag="ps"
                        )
                    for kk in range(KT):
                        for bb in range(BT):
                            nc.tensor.matmul(
                                ps[bb][g][:],
                                wt[:, kk, g * 128:(g + 1) * 128],
                                at[:, kk, bb * 512:(bb + 1) * 512],
                                start=(kk == 0),
                                stop=(kk == KT - 1),
                            )

                bcol = lambda g: bias_t[:, hh * G + g:hh * G + g + 1]
                last_tile = hh == HT - 1

                def pre(g, bb, name):
                    # pre-activation = psum + neighbor term (bias joins via
                    # the ACT bias port)
                    t = p_eps.tile([128, 512], f32, name=name, tag=f"{name}{bb}")
                    nc.vector.tensor_add(t[:], ps[bb][g][:], nbts[bb][:, g, :])
                    return t

                # bb0/bb1 interleaved per gate: DVE/ACT stay FIFO-pipelined
                # and each psum bank frees as early as its data allows.
                tan_s = [pre(3, bb, "tan_s") for bb in range(BT)]
                for bb in range(BT):
                    nc.scalar.activation(tan_s[bb][:], tan_s[bb][:], ACT.Tanh,
                                         bias=bcol(3))
                sig_i = [pre(0, bb, "sig_i") for bb in range(BT)]
                for bb in range(BT):
                    nc.scalar.activation(sig_i[bb][:], sig_i[bb][:], ACT.Sigmoid,
                                         bias=bcol(0))
                sig_f = [pre(1, bb, "sig_f") for bb in range(BT)]
                for bb in range(BT):
                    nc.scalar.activation(sig_f[bb][:], sig_f[bb][:], ACT.Sigmoid,
                                         bias=bcol(1))

                c_news, tan_cs = [], []
                for bb in range(BT):
                    t_is = p_eps.tile([128, 512], f32, name="t_is", tag=f"t_is{bb}")
                    nc.vector.tensor_mul(t_is[:], sig_i[bb][:], tan_s[bb][:])
                    t_fc = p_eps.tile([128, 512], f32, name="t_fc", tag=f"t_fc{bb}")
                    nc.vector.tensor_mul(t_fc[:], sig_f[bb][:], cts[bb][:])
                    c_new = p_out.tile([128, 512], f16, name="c_new", tag=f"c_new{bb}")
                    nc.vector.tensor_add(c_new[:], t_is[:], t_fc[:])
                    c_news.append(c_new)
                    tan_c = p_eps.tile([128, 512], f32, name="tan_c", tag=f"tan_c{bb}")
                    # explicit zero-bias AP: a float bias would make the
                    # framework stage a const tensor via a TENSOR_LOAD that
                    # delays the sync queue's first DMA at the critical head
                    nc.scalar.activation(tan_c[:], c_new[:], ACT.Tanh,
                                         bias=bias_t[:, HT * G:HT * G + 1])
                    tan_cs.append(tan_c)
                    # c outputs are ready before the o-gate stream ends; the
                    # last tile's ride the sync queue (idle by then) so their
                    # issue cost doesn't delay the scalar queue's sig_o
                    qc = nc.sync if last_tile else nc.gpsimd
                    qc.dma_start(co_d[hh, bb][:], c_new[:])

                if not last_tile:
                    sig_o = [pre(2, bb, "sig_o") for bb in range(BT)]
                    for bb in range(BT):
                        nc.scalar.activation(sig_o[bb][:], sig_o[bb][:],
                                             ACT.Sigmoid, bias=bcol(2))
                    for bb in range(BT):
                        h_new = p_out.tile([128, 512], f16, name="h_new",
                                           tag=f"h_new{bb}")
                        nc.vector.tensor_mul(h_new[:], sig_o[bb][:], tan_cs[bb][:])
                        nc.gpsimd.dma_start(ho_d[hh, bb][:], h_new[:])
                else:
                    # tail: the o-gate path gates the end of the kernel, so
                    # run it in column halves -- each half's h output flushes
                    # (on the idle sync queue) while the next half computes
                    for bb in range(BT):
                        for lo, hi in ((0, 256), (256, 512)):
                            sl = slice(lo, hi)
                            t = p_eps.tile([128, hi - lo], f32, name="sig_o",
                                           tag=f"sig_o{bb}_{lo}")
                            nc.vector.tensor_add(t[:], ps[bb][2][:, sl],
                                                 nbts[bb][:, 2, sl])
                            nc.scalar.activation(t[:], t[:], ACT.Sigmoid,
                                                 bias=bcol(2))
                            h_new = p_out.tile([128, hi - lo], f16, name="h_new",
                                               tag=f"h_new{bb}_{lo}")
                            nc.vector.tensor_mul(h_new[:], t[:], tan_cs[bb][:, sl])
                            nc.sync.dma_start(ho_d[hh, bb][:, sl], h_new[:])

    _dedupe_ldweights(nc)
    _split_excess_waits(nc)
    return nc


def _get_program():
    global _PROG
    if _PROG is None:
        _PROG = _build_program()
    return _PROG


def _prep_inputs(x, hidden, cell, neighbors, Wx, Wh, Wn, b):
    """Host-side shard/relayout. Returns per-core input maps."""
    x = np.asarray(x, np.float32)
    hidden = np.asarray(hidden, np.float32)
    cell = np.asarray(cell, np.float32)
    neighbors = np.asarray(neighbors, np.float32)
    Wx = np.asarray(Wx, np.float32)
    Wh = np.asarray(Wh, np.float32)
    Wn = np.asarray(Wn, np.float32)
    b = np.asarray(b, np.float32)

    # A = [x | hidden]: K = 2048 exactly.
    A = np.concatenate([x, hidden], axis=1)
    W_all = np.concatenate([Wx, Wh], axis=2)  # [G, H, 2048]

    # SBUF weight layout: [hh, p(k), kk, g*128 + j(h)], fp16
    w_host = np.ascontiguousarray(
        W_all.reshape(G, HT, 128, KT, 128).transpose(1, 4, 3, 0, 2)
    ).reshape(HT, 128, KT, G * 128).astype(np.float16)

    # neighbor term, [B, G, H] computed on host in f64 -> f32
    nbterm = np.einsum(
        "bj,ghj->gbh", neighbors.astype(np.float64), Wn.astype(np.float64)
    ).astype(np.float32)

    # bias layout [j, hh*G + g] = b[g, hh*128+j]
    bias_host = np.zeros((128, HT * G + 1), np.float32)
    bias_host[:, :HT * G] = b.reshape(G, HT, 128).transpose(2, 1, 0).reshape(128, HT * G)

    in_maps = []
    for c in range(NCORES):
        sl = slice(c * BS, (c + 1) * BS)
        # A^T tiled: [p(k), kk, b], fp16
        at_host = np.ascontiguousarray(
            A[sl].T.reshape(KT, 128, BS).transpose(1, 0, 2)
        ).astype(np.float16)
        # cell^T tiled: [hh, bb, j(h), n(b)], fp16
        ct_host = np.ascontiguousarray(
            cell[sl].T.reshape(HT, 128, BT, 512).transpose(0, 2, 1, 3)
        ).astype(np.float16)
        # neighbor term tiled: [hh, bb, j(h), g, n(b)], fp16
        nb_host = np.ascontiguousarray(
            nbterm[:, sl, :].transpose(2, 1, 0)  # [H, BS, G]
            .reshape(HT, 128, BT, 512, G)
            .transpose(0, 2, 1, 4, 3)            # [hh, bb, j, g, n]
        ).astype(np.float16)
        in_maps.append(
            {
                "AT": at_host,
                "W": w_host,
                "CT": ct_host,
                "NBT": nb_host,
                "BIAS": bias_host,
            }
        )
    return in_maps


def _gather_outputs(results):
    """Invert the per-core [HT, BT, 128, 512] transposed tiling."""
    h_parts, c_parts = [], []
    for c in range(NCORES):
        hT = np.asarray(results[c]["hT"]).astype(np.float32)
        cT = np.asarray(results[c]["cT"]).astype(np.float32)
        # [hh, bb, j, n] -> [hh*128+j, bb*512+n] -> transpose to [b, h]
        h_parts.append(hT.transpose(0, 2, 1, 3).reshape(H, BS).T)
        c_parts.append(cT.transpose(0, 2, 1, 3).reshape(H, BS).T)
    next_hidden = np.ascontiguousarray(np.concatenate(h_parts, axis=0), dtype=np.float32)
    next_cell = np.ascontiguousarray(np.concatenate(c_parts, axis=0), dtype=np.float32)
    return next_hidden, next_cell


def _run(in_maps, trace=False, tmpdir=None):
    _import_concourse()
    from concourse.bass_utils import run_bass_kernel_spmd

    if trace:
        _install_ntff_shim()
    nc = _get_program()
    last_err = None
    for attempt in range(3):
        try:
            return run_bass_kernel_spmd(
                nc, in_maps, list(range(NCORES)), trace=trace, tmpdir=tmpdir
            )
        except Exception as e:  # transient device wedge: retry
            last_err = e
            if "UNRECOVERABLE" not in str(e) and "NRT" not in str(e):
                raise
    raise last_err


def _install_ntff_shim():
    """Shim antenv.axon_hooks (absent in this image) so trace=True works."""
    import types

    if "antenv.axon_hooks" not in sys.modules:
        mod = types.ModuleType("antenv.axon_hooks")
        mod._hook = None
        mod.set_axon_ntff_profile_hook = lambda h: setattr(mod, "_hook", h)
        mod.get_axon_ntff_profile_hook = lambda: mod._hook
        sys.modules["antenv.axon_hooks"] = mod
        try:
            import antenv
            antenv.axon_hooks = mod
        except ImportError:
            pass
    mod = sys.modules["antenv.axon_hooks"]
    if mod._hook is None:
        from trn_agent_boot.trn_boot import _ntff_profile_via_ctypes
        mod._hook = _ntff_profile_via_ctypes("/opt/axon/libaxon_pjrt.so")
    from concourse import bass_utils
    bass_utils.upload_artifacts = lambda tmpdir: f"local:{tmpdir}"


def kernel(x, hidden, cell, neighbors, Wx, Wh, Wn, b):
    _import_concourse()
    in_maps = _prep_inputs(x, hidden, cell, neighbors, Wx, Wh, Wn, b)
    res = _run(in_maps, trace=False)
    return _gather_outputs(res.results)



# revision 32
# speedup vs baseline: 1.0343x; 1.0152x over previous
"""Trainium2 Bass kernel for nn_BiLSTMCell (graph-LSTM cell).

Math (per batch row):
    g_pre[g] = x @ Wx[g].T + hidden @ Wh[g].T + neighbors @ Wn[g].T + b[g]
    i, f, o = sigmoid(g_pre[0..2]);  s = tanh(g_pre[3])
    next_cell = f * cell + i * s
    next_hidden = o * tanh(next_cell)

Strategy: data-parallel over the batch (8192 -> 1024 rows/core on 8 cores),
weights replicated. The x/hidden operands are fused on host into one
A = [x | hidden] with K = 2048 = 16*128, so each gate pre-activation is a
single 16-step accumulating PE matmul chain:
    g_pre[g]^T = W_all[g] @ A^T      ([128k,128h]^T @ [128k,512b] per step)
in fp16 (f32 PSUM accumulate; fp16 enables Fast Weight Load so the
128-cycle LDWEIGHTS hides under the 512-cycle stream, unlike f32r which
paid it serially -- ~280ns/matmul -> ~220ns/matmul).

The rank-4 neighbor term (neighbors @ Wn[g].T, 0.27 GFLOP) is computed on
the host and shipped as an f32 addend; it joins the pre-activation via one
VectorE add per gate. The bias rides the ScalarE activation's per-partition
bias port for free. This keeps the PE stream at exactly 1024 matmuls/core.

Outputs are produced transposed/tiled and unscrambled on the host.
"""

import os
import sys

import numpy as np


def _import_concourse():
    try:
        import concourse.bass  # noqa: F401
        return
    except ImportError:
        pass
    for p in ("/opt/trn_rl_repo", "/root/.axon_site/_ro/trn_rl_repo"):
        if os.path.isdir(p) and p not in sys.path:
            sys.path.insert(0, p)
    import concourse.bass  # noqa: F401


B, IN, H, NB, G = 8192, 1024, 1024, 4, 4
NCORES = 8
BS = B // NCORES        # 1024 batch rows per core
KT = 16                 # k-tiles of 128 (IN + H = 2048)
HT = H // 128           # 8 h-tiles of 128
BT = BS // 512          # 2 b-tiles of 512


def _split_excess_waits(nc, max_waits=1, drain_max=0):
    """This walrus build's codegen supports very few sync-wait commands per
    instruction (1 for most ops, 0 spare on Drain). Hoist excess sem-waits
    onto preceding wait-only NoOps on the same engine (AND-semantics over
    monotone semaphores makes sequential waiting equivalent)."""
    from concourse import mybir

    uid = [0]
    n_split = 0
    for fn in nc.m.functions:
        for bb in fn.blocks:
            new_insts = []
            for inst in bb.instructions:
                limit = drain_max if type(inst).__name__ == "InstDrain" else max_waits
                si = inst.sync_info
                waits = list(si.on_wait) if si and si.on_wait else []
                if len(waits) > limit:
                    n_split += 1
                    if limit > 0:
                        excess, keep = waits[:-limit], waits[-limit:]
                    else:
                        excess, keep = waits, []
                    for i in range(0, len(excess), max_waits):
                        chunk = excess[i:i + max_waits]
                        nop = mybir.InstNoOp(
                            name=f"waitsplit_{uid[0]}",
                            sync_info=mybir.SyncInfo(on_wait=chunk, on_update=[]),
                        )
                        uid[0] += 1
                        nop.engine = inst.engine
                        new_insts.append(nop)
                    si.on_wait = keep
                    inst.sync_info = si
                new_insts.append(inst)
            bb.instructions = new_insts
    return n_split


def _dedupe_ldweights(nc):
    """Our bb-paired emission produces [LDW_a, MM(b0), LDW_b, MM(b1)] with
    LDW_a == LDW_b (identical weight AP). The PE only commits freshly loaded
    weights after the in-flight matmul fully drains (~110 cycles), so every
    redundant reload costs ~46ns. Deleting LDW_b lets MM(b1) stream against
    the already-committed weights back-to-back. Waits on a deleted LDW are
    merged into the following matmul (excess waits are split later by
    _split_excess_waits)."""
    from concourse import mybir

    n_del = 0
    for fn in nc.m.functions:
        for bb in fn.blocks:
            insts = bb.instructions
            new_insts = []
            last_ldw_key = None
            pending_waits = []
            for inst in insts:
                tname = type(inst).__name__
                if tname == "InstLdweights":
                    key = str(inst.ins[0])
                    if key == last_ldw_key:
                        si = inst.sync_info
                        if si and si.on_wait:
                            pending_waits.extend(si.on_wait)
                        n_del += 1
                        continue
                    last_ldw_key = key
                    new_insts.append(inst)
                elif tname == "InstMatmult":
                    if pending_waits:
                        si = inst.sync_info or mybir.SyncInfo(
                            on_wait=[], on_update=[]
                        )
                        si.on_wait = list(si.on_wait) + pending_waits
                        inst.sync_info = si
                        pending_waits = []
                    new_insts.append(inst)
                else:
                    # any other PE-visible instruction invalidates the cache
                    if getattr(inst, "engine", None) == mybir.EngineType.PE:
                        last_ldw_key = None
                    new_insts.append(inst)
            assert not pending_waits
            bb.instructions = new_insts
    return n_del


_PROG = None


def _build_program():
    import concourse.bass as bass
    import concourse.tile as tile
    from concourse import mybir

    f32 = mybir.dt.float32
    f16 = mybir.dt.float16
    ACT = mybir.ActivationFunctionType

    nc = bass.Bass()
    at_d = nc.dram_tensor("AT", [128, KT, BS], f16, kind="ExternalInput")
    w_d = nc.dram_tensor("W", [HT, 128, KT, G * 128], f16, kind="ExternalInput")
    ct_d = nc.dram_tensor("CT", [HT, BT, 128, 512], f16, kind="ExternalInput")
    nb_d = nc.dram_tensor("NBT", [HT, BT, 128, G, 512], f16, kind="ExternalInput")
    bias_d = nc.dram_tensor("BIAS", [128, HT * G + 1], f32, kind="ExternalInput")
    ho_d = nc.dram_tensor("hT", [HT, BT, 128, 512], f16, kind="ExternalOutput")
    co_d = nc.dram_tensor("cT", [HT, BT, 128, 512], f16, kind="ExternalOutput")

    with tile.TileContext(nc) as tc:
        with (
            tc.tile_pool(name="at", bufs=1) as p_at,
            tc.tile_pool(name="w", bufs=2) as p_w,
            tc.tile_pool(name="cell", bufs=3) as p_cell,
            tc.tile_pool(name="nb", bufs=2) as p_nb,
            tc.tile_pool(name="bias", bufs=1) as p_bias,
            tc.tile_pool(name="eps", bufs=2) as p_eps,
            tc.tile_pool(name="outs", bufs=2) as p_out,
            tc.tile_pool(name="ps", bufs=8, space="PSUM") as p_ps,
        ):
            bias_t = p_bias.tile([128, HT * G + 1], f32, name="bias_t")
            nc.gpsimd.dma_start(bias_t[:], bias_d[:])
            at = p_at.tile([128, KT, BS], f16, name="at")

            # One queue saturates HBM (~390 GB/s), so multi-queue splits only
            # reduce each stream's share. Put ALL bulk traffic (W, AT) on the
            # sync queue in exact consumption order; cell/neighbor ride the
            # scalar queue, outputs get gpsimd to themselves.
            wts = []
            for hh in range(HT):
                wts.append(p_w.tile([128, KT, G * 128], f16, name="wt", tag="wt"))

            # head: every dma_start costs ~0.8us of SEQUENCER issue time
            # (DIRECT2D descriptor gen), so the head is issue-rate limited,
            # not bandwidth limited. Split W0 (sync queue) and AT (scalar
            # queue) so the two issue streams run in parallel, and coarsen
            # chunks progressively: small chunks up front so the PE starts
            # at ~8us, big chunks behind to cut issue count.
            w0_chunks = [(0, 1), (1, 2), (2, 3), (3, 4),
                         (4, 6), (6, 8), (8, 12), (12, 16)]
            at_chunks = [(0, 1), (1, 2), (2, 4), (4, 6),
                         (6, 8), (8, 12), (12, 16)]
            for lo, hi in w0_chunks:
                nc.sync.dma_start(wts[0][:, lo:hi, :], w_d[0, :, lo:hi, :])
            for lo, hi in at_chunks:
                nc.scalar.dma_start(at[:, lo:hi, :], at_d[:, lo:hi, :])
            for hh in range(1, HT):
                nc.sync.dma_start(wts[hh][:], w_d[hh])

            for hh in range(HT):
                wt = wts[hh]

                cts, nbts = [], []
                for bb in range(BT):
                    ct = p_cell.tile([128, 512], f16, name="ct", tag="ct")
                    nc.scalar.dma_start(ct[:], ct_d[hh, bb])
                    cts.append(ct)
                    nbt = p_nb.tile([128, G, 512], f16, name="nbt", tag="nbt")
                    nc.scalar.dma_start(nbt[:], nb_d[hh, bb])
                    nbts.append(nbt)

                # Both b-tiles processed together so each weight tile is
                # loaded ONCE and streamed against b0 then b1: the second
                # matmul of a pair needs no weight-slot commit, so its fill
                # overlaps the first's drain (weight commit requires a fully
                # drained array -> 259 ns/mm when every mm reloads weights).
                # Gates run as sequential blocks (s,i,f,o) so the s-block's
                # psum banks free ~75% before the h-tile's stream ends and
                # the next h-tile never waits on bank recycling.
                ps = [[None] * G for _ in range(BT)]
                for g in (3, 0, 1, 2):
                    for bb in range(BT):
                        ps[bb][g] = p_ps.tile(
                            [128, 512], f32, name=f"pt{g}_{bb}", tag="ps"
                        )
                    for kk in range(KT):
                        for bb in range(BT):
                            nc.tensor.matmul(
                                ps[bb][g][:],
                                wt[:, kk, g * 128:(g + 1) * 128],
                                at[:, kk, bb * 512:(bb + 1) * 512],
                                start=(kk == 0),
                                stop=(kk == KT - 1),
                            )

                bcol = lambda g: bias_t[:, hh * G + g:hh * G + g + 1]
                last_tile = hh == HT - 1

                def pre(g, bb, name):
                    # pre-activation = psum + neighbor term (bias joins via
                    # the ACT bias port)
                    t = p_eps.tile([128, 512], f32, name=name, tag=f"{name}{bb}")
                    nc.vector.tensor_add(t[:], ps[bb][g][:], nbts[bb][:, g, :])
                    return t

                # bb0/bb1 interleaved per gate: DVE/ACT stay FIFO-pipelined
                # and each psum bank frees as early as its data allows.
                tan_s = [pre(3, bb, "tan_s") for bb in range(BT)]
                for bb in range(BT):
                    nc.scalar.activation(tan_s[bb][:], tan_s[bb][:], ACT.Tanh,
                                         bias=bcol(3))
                sig_i = [pre(0, bb, "sig_i") for bb in range(BT)]
                for bb in range(BT):
                    nc.scalar.activation(sig_i[bb][:], sig_i[bb][:], ACT.Sigmoid,
                                         bias=bcol(0))
                sig_f = [pre(1, bb, "sig_f") for bb in range(BT)]
                for bb in range(BT):
                    nc.scalar.activation(sig_f[bb][:], sig_f[bb][:], ACT.Sigmoid,
                                         bias=bcol(1))

                c_news, tan_cs = [], []
                for bb in range(BT):
                    t_is = p_eps.tile([128, 512], f32, name="t_is", tag=f"t_is{bb}")
                    nc.vector.tensor_mul(t_is[:], sig_i[bb][:], tan_s[bb][:])
                    t_fc = p_eps.tile([128, 512], f32, name="t_fc", tag=f"t_fc{bb}")
                    nc.vector.tensor_mul(t_fc[:], sig_f[bb][:], cts[bb][:])
                    c_new = p_out.tile([128, 512], f16, name="c_new", tag=f"c_new{bb}")
                    nc.vector.tensor_add(c_new[:], t_is[:], t_fc[:])
                    c_news.append(c_new)
                    tan_c = p_eps.tile([128, 512], f32, name="tan_c", tag=f"tan_c{bb}")
                    # explicit zero-bias AP: a float bias would make the
                    # framework stage a const tensor via a TENSOR_LOAD that
                    # delays the sync queue's first DMA at the critical head
                    nc.scalar.activation(tan_c[:], c_new[:], ACT.Tanh,
                                         bias=bias_t[:, HT * G:HT * G + 1])
                    tan_cs.append(tan_c)
                    # c outputs are ready before the o-gate stream ends; the
                    # last tile's ride the sync queue (idle by then) so their
                    # issue cost doesn't delay the scalar queue's sig_o
                    qc = nc.sync if last_tile else nc.gpsimd
                    qc.dma_start(co_d[hh, bb][:], c_new[:])

                if not last_tile:
                    sig_o = [pre(2, bb, "sig_o") for bb in range(BT)]
                    for bb in range(BT):
                        nc.scalar.activation(sig_o[bb][:], sig_o[bb][:],
                                             ACT.Sigmoid, bias=bcol(2))
                    for bb in range(BT):
                        h_new = p_out.tile([128, 512], f16, name="h_new",
                                           tag=f"h_new{bb}")
                        nc.vector.tensor_mul(h_new[:], sig_o[bb][:], tan_cs[bb][:])
                        nc.gpsimd.dma_start(ho_d[hh, bb][:], h_new[:])
                else:
                    # tail: the o-gate path gates the end of the kernel, so
                    # run it in column halves -- each half's h output flushes
                    # (on the idle sync queue) while the next half computes
                    for bb in range(BT):
                        for lo, hi in ((0, 256), (256, 512)):
                            sl = slice(lo, hi)
                            t = p_eps.tile([128, hi - lo], f32, name="sig_o",
                                           tag=f"sig_o{bb}_{lo}")
                            nc.vector.tensor_add(t[:], ps[bb][2][:, sl],
                                                 nbts[bb][:, 2, sl])
                            nc.scalar.activation(t[:], t[:], ACT.Sigmoid,
                                                 bias=bcol(2))
                            h_new = p_out.tile([128, hi - lo], f16, name="h_new",
                                               tag=f"h_new{bb}_{lo}")
                            nc.vector.tensor_mul(h_new[:], t[:], tan_cs[bb][:, sl])
                            nc.sync.dma_start(ho_d[hh, bb][:, sl], h_new[:])

    _dedupe_ldweights(nc)
    _split_excess_waits(nc)
    return nc


def _get_program():
    global _PROG
    if _PROG is None:
        _PROG = _build_program()
    return _PROG


def _prep_inputs(x, hidden, cell, neighbors, Wx, Wh, Wn, b):
    """Host-side shard/relayout. Returns per-core input maps."""
    x = np.asarray(x, np.float32)
    hidden = np.asarray(hidden, np.float32)
    cell = np.asarray(cell, np.float32)
    neighbors = np.asarray(neighbors, np.float32)
    Wx = np.asarray(Wx, np.float32)
    Wh = np.asarray(Wh, np.float32)
    Wn = np.asarray(Wn, np.float32)
    b = np.asarray(b, np.float32)

    # A = [x | hidden]: K = 2048 exactly.
    A = np.concatenate([x, hidden], axis=1)
    W_all = np.concatenate([Wx, Wh], axis=2)  # [G, H, 2048]

    # SBUF weight layout: [hh, p(k), kk, g*128 + j(h)], fp16
    w_host = np.ascontiguousarray(
        W_all.reshape(G, HT, 128, KT, 128).transpose(1, 4, 3, 0, 2)
    ).reshape(HT, 128, KT, G * 128).astype(np.float16)

    # neighbor term, [B, G, H] computed on host in f64 -> f32
    nbterm = np.einsum(
        "bj,ghj->gbh", neighbors.astype(np.float64), Wn.astype(np.float64)
    ).astype(np.float32)

    # bias layout [j, hh*G + g] = b[g, hh*128+j]
    bias_host = np.zeros((128, HT * G + 1), np.float32)
    bias_host[:, :HT * G] = b.reshape(G, HT, 128).transpose(2, 1, 0).reshape(128, HT * G)

    in_maps = []
    for c in range(NCORES):
        sl = slice(c * BS, (c + 1) * BS)
        # A^T tiled: [p(k), kk, b], fp16
        at_host = np.ascontiguousarray(
            A[sl].T.reshape(KT, 128, BS).transpose(1, 0, 2)
        ).astype(np.float16)
        # cell^T tiled: [hh, bb, j(h), n(b)], fp16
        ct_host = np.ascontiguousarray(
            cell[sl].T.reshape(HT, 128, BT, 512).transpose(0, 2, 1, 3)
        ).astype(np.float16)
        # neighbor term tiled: [hh, bb, j(h), g, n(b)], fp16
        nb_host = np.ascontiguousarray(
            nbterm[:, sl, :].transpose(2, 1, 0)  # [H, BS, G]
            .reshape(HT, 128, BT, 512, G)
            .transpose(0, 2, 1, 4, 3)            # [hh, bb, j, g, n]
        ).astype(np.float16)
        in_maps.append(
            {
                "AT": at_host,
                "W": w_host,
                "CT": ct_host,
                "NBT": nb_host,
                "BIAS": bias_host,
            }
        )
    return in_maps


def _gather_outputs(results):
    """Invert the per-core [HT, BT, 128, 512] transposed tiling."""
    h_parts, c_parts = [], []
    for c in range(NCORES):
        hT = np.asarray(results[c]["hT"]).astype(np.float32)
        cT = np.asarray(results[c]["cT"]).astype(np.float32)
        # [hh, bb, j, n] -> [hh*128+j, bb*512+n] -> transpose to [b, h]
        h_parts.append(hT.transpose(0, 2, 1, 3).reshape(H, BS).T)
        c_parts.append(cT.transpose(0, 2, 1, 3).reshape(H, BS).T)
    next_hidden = np.ascontiguousarray(np.concatenate(h_parts, axis=0), dtype=np.float32)
    next_cell = np.ascontiguousarray(np.concatenate(c_parts, axis=0), dtype=np.float32)
    return next_hidden, next_cell


def _run(in_maps, trace=False, tmpdir=None):
    _import_concourse()
    from concourse.bass_utils import run_bass_kernel_spmd

    if trace:
        _install_ntff_shim()
    nc = _get_program()
    last_err = None
    for attempt in range(3):
        try:
            return run_bass_kernel_spmd(
                nc, in_maps, list(range(NCORES)), trace=trace, tmpdir=tmpdir
            )
        except Exception as e:  # transient device wedge: retry
            last_err = e
            if "UNRECOVERABLE" not in str(e) and "NRT" not in str(e):
                raise
    raise last_err


def _install_ntff_shim():
    """Shim antenv.axon_hooks (absent in this image) so trace=True works."""
    import types

    if "antenv.axon_hooks" not in sys.modules:
        mod = types.ModuleType("antenv.axon_hooks")
        mod._hook = None
        mod.set_axon_ntff_profile_hook = lambda h: setattr(mod, "_hook", h)
        mod.get_axon_ntff_profile_hook = lambda: mod._hook
        sys.modules["antenv.axon_hooks"] = mod
        try:
            import antenv
            antenv.axon_hooks = mod
        except ImportError:
            pass
    mod = sys.modules["antenv.axon_hooks"]
    if mod._hook is None:
        from trn_agent_boot.trn_boot import _ntff_profile_via_ctypes
        mod._hook = _ntff_profile_via_ctypes("/opt/axon/libaxon_pjrt.so")
    from concourse import bass_utils
    bass_utils.upload_artifacts = lambda tmpdir: f"local:{tmpdir}"


def kernel(x, hidden, cell, neighbors, Wx, Wh, Wn, b):
    _import_concourse()
    in_maps = _prep_inputs(x, hidden, cell, neighbors, Wx, Wh, Wn, b)
    res = _run(in_maps, trace=False)
    return _gather_outputs(res.results)

